# revision 10
# baseline (speedup 1.0000x reference)
"""Chamfer loss kernel for Trainium2 (Bass/Tile), 8 NeuronCores.

Math: for each batch b, D_b[n, m] = ||pred[b,n] - label[b,m]||.
result = mean_n(min_m D) + mean_m(min_n D).

Strategy
--------
Sharding: 8 cores = 4 batches x 2 halves of the pred axis. Core c
(b = c//2, h = c%2) owns queries q = pred[b, h*4096:(h+1)*4096] (NQ=4096)
and all refs r = label[b] (NR=8192). Each core makes ONE pass over its
4096 x 8192 block of the (negated) squared-distance matrix and produces
BOTH reductions from that single pass:
  - per-ref  max of -d^2 over its 4096 queries -> partial min_n; the two
    halves of a batch are combined on the host (tiny elementwise max).
  - per-query max of -d^2 over all 8192 refs -> complete min_m for its
    4096 pred points (finished on device: relu, sqrt, partial sums).

PE: -d^2 = 2 q.r - ||q||^2 - ||r||^2 as a K=16 bf16 matmul using the
split-bf16 trick (q ~ qh+ql, r ~ rh+rl, norms split hi/lo as well), so
products are exact bf16xbf16 accumulated in fp32 -> ~fp32 accuracy at
1 cycle/row (4x faster than the fp32 PE path). Stationary = 128 refs
per row tile, moving = 512 queries per matmul.

Consume per [128, 2048] PSUM group:
  1. copy+downcast PSUM -> fp16 SBUF tile (ScalarE mostly — it is
     otherwise idle — DVE for a tunable fraction),
  2. DVE reduce_max of the fp16 tile (fast 2-byte SBUF mode) chained
     into the per-ref RMS[:, r],
  3. elementwise fp16 max into the per-query accumulator CM (GPSIMD /
     DVE split, tunable).
Tail: clamp+convert CM -> fp32, PE-transpose 128x128 blocks into PSUM,
one 3D reduce_max over the candidate axis, sqrt, partial sums.

kernel(pred, label) takes the full inputs, shards on host (layout +
tiny O(N*D) augmentation only), runs the SPMD program on cores 0-7 via
run_bass_kernel_spmd, and combines the small per-core outputs.
"""

import os
import sys

import numpy as np

for _p in ("/opt/trn_rl_repo", "/root/.axon_site/_ro/trn_rl_repo"):
    if os.path.isdir(_p) and _p not in sys.path:
        sys.path.append(_p)

import ml_dtypes

import concourse.bacc as bacc
import concourse.mybir as mybir
from concourse import tile
from concourse.bass_utils import run_bass_kernel_spmd

F32 = mybir.dt.float32
F16 = mybir.dt.float16
BF16 = mybir.dt.bfloat16
NPBF16 = ml_dtypes.bfloat16
OP_MAX = mybir.AluOpType.max
AX_X = mybir.AxisListType.X
SQRT = mybir.ActivationFunctionType.Sqrt
COPY = mybir.ActivationFunctionType.Copy

B = 4
N = 8192
NCORES = 8
NEG16 = -60000.0

# full-size kernel geometry
NQ = N // 2      # queries per core (pred half)
NR = N           # refs per core (all labels of the batch)
MMN = 512        # moving free dim per matmul (one PSUM bank)
K = 16           # split-bf16 augmented contraction dim


def build_program(nq=NQ, nr=NR, mmn=MMN, copy_dve_every=10, fold_dve_every=1):
    """Emit + compile the per-core program.

    copy_dve_every: every n-th PSUM->fp16 copy runs on DVE instead of ACT
      (0 = all on ACT). Balances the two engines' PSUM read load.
    fold_dve_every: every n-th CM fold runs on DVE (1 = all on DVE; the
    Pool engine has no TensorTensor max ucode, so the rest would need
    another mechanism — currently everything lands on DVE).
    """
    nchunk = 4 * mmn               # columns per consume group
    ngroup = nq // nchunk          # consume groups per ref row-tile
    rt = nr // 128                 # ref row-tiles
    assert nq % nchunk == 0 and nr % 128 == 0 and nq % 128 == 0

    nc = bacc.Bacc("TRN2", target_bir_lowering=False, debug=False)
    qs_d = nc.dram_tensor("qs", [K, nq], BF16, kind="ExternalInput")
    rs_d = nc.dram_tensor("rs", [K, nr], BF16, kind="ExternalInput")
    id_d = nc.dram_tensor("ident", [128, 128], F32, kind="ExternalInput")
    refout_d = nc.dram_tensor("ref_out", [128, rt], F32, kind="ExternalOutput")
    qout_d = nc.dram_tensor("q_out", [128, 1], F32, kind="ExternalOutput")

    with tile.TileContext(nc) as tc:
        with (
            tc.tile_pool(name="const", bufs=1) as const,
            tc.tile_pool(name="rmp", bufs=2) as rmp,
            tc.tile_pool(name="scp", bufs=4) as scp,
            tc.tile_pool(name="tail", bufs=1) as tail,
        ):
            RS = const.tile([K, nr], BF16)
            nc.sync.dma_start(RS[:], rs_d.ap())
            QS = const.tile([K, nq], BF16)
            nc.sync.dma_start(QS[:], qs_d.ap())
            IDENT = const.tile([128, 128], F32)
            nc.sync.dma_start(IDENT[:], id_d.ap())
            CM = const.tile([128, nq], F16)
            nc.vector.memset(CM[:], NEG16)
            RMS = const.tile([128, rt], F32)

            gi = 0  # global group index
            main_psum = tc.tile_pool(name="psum", bufs=2, space="PSUM")
            psum = main_psum.__enter__()
            for r in range(rt):
                lhs = RS[:, r * 128:(r + 1) * 128]
                acc = None
                for j2 in range(ngroup):
                    ps = psum.tile([128, nchunk], F32)
                    for i in range(4):
                        c = j2 * 4 + i
                        nc.tensor.matmul(
                            ps[:, i * mmn:(i + 1) * mmn],
                            lhs,
                            QS[:, c * mmn:(c + 1) * mmn],
                            start=True,
                            stop=True,
                        )
                    sc = scp.tile([128, nchunk], F16, tag="sc")
                    if copy_dve_every and gi % copy_dve_every == 0:
                        nc.vector.tensor_copy(sc[:], ps[:])
                    else:
                        nc.scalar.activation(sc[:], ps[:], COPY)
                    # per-ref reduce (chained across groups at row-tile end)
                    if ngroup == 1:
                        nc.vector.reduce_max(RMS[:, r:r + 1], sc[:], axis=AX_X)
                    else:
                        rg = rmp.tile([128, 1], F32, tag=f"rg{j2 % 2}")
                        nc.vector.reduce_max(rg[:], sc[:], axis=AX_X)
                        if acc is None:
                            acc = rg
                        elif j2 == ngroup - 1:
                            nc.vector.tensor_max(RMS[:, r:r + 1], acc[:], rg[:])
                        else:
                            nacc = rmp.tile([128, 1], F32, tag="racc")
                            nc.vector.tensor_max(nacc[:], acc[:], rg[:])
                            acc = nacc
                    # per-query fold into CM
                    cm_sl = CM[:, j2 * nchunk:(j2 + 1) * nchunk]
                    nc.vector.tensor_max(cm_sl, cm_sl, sc[:])
                    gi += 1

            nc.sync.dma_start(refout_d.ap(), RMS[:])
            main_psum.__exit__(None, None, None)

            # per-query direction: max over the 128 partitions of CM.
            # Clamp+convert CM16 -> fp32, PE-transpose each 128x128 block
            # into PSUM, then one reduce_max over the (now innermost-free)
            # candidate axis, then sqrt of the negated minima and sum.
            CM32 = tail.tile([128, nq], F32)
            nc.vector.tensor_scalar_min(CM32[:], CM[:], 0.0)
            with tc.tile_pool(name="psum2", bufs=1, space="PSUM") as psum2:
                pst = psum2.tile([128, nq], F32)
                nblk = nq // 128
                for blk in range(nblk):
                    nc.tensor.transpose(
                        pst[:, blk * 128:(blk + 1) * 128],
                        CM32[:, blk * 128:(blk + 1) * 128],
                        IDENT[:],
                    )
                q2 = tail.tile([128, nblk], F32)
                nc.vector.tensor_reduce(
                    q2[:], pst[:].rearrange("p (b c) -> p b c", c=128),
                    axis=AX_X, op=OP_MAX,
                )
            # q2 holds v = max(-d^2) clamped <= 0; sqrt(-v) = distance.
            sq = tail.tile([128, nblk], F32)
            nc.scalar.activation(sq[:], q2[:], SQRT, bias=0.0, scale=-1.0)
            qsum = tail.tile([128, 1], F32)
            nc.vector.reduce_sum(qsum[:], sq[:], axis=AX_X)
            nc.sync.dma_start(qout_d.ap(), qsum[:])

    nc.compile()
    return nc


def _split2(x):
    """fp32 -> (hi, lo) fp32 arrays exactly representable in bf16."""
    hi = x.astype(NPBF16).astype(np.float32)
    lo = (x - hi).astype(NPBF16).astype(np.float32)
    return hi, lo


def prep_core(q, r, mmn=MMN):
    """Build the split-bf16 augmented layouts for one core.

    dot(qaug[:, n], raug[:, m]) = 2 qt.rt - ||qt||^2 - ||rt||^2
                                = -||qt - rt||^2
    with qt = qh+ql (~fp32 accurate), rt = rh+rl.
    """
    nq, nr = q.shape[0], r.shape[0]
    q = np.ascontiguousarray(q, np.float32)
    r = np.ascontiguousarray(r, np.float32)
    qh, ql = _split2(q)
    rh, rl = _split2(r)
    qt = qh + ql
    rtt = rh + rl
    q2h, q2l = _split2((qt * qt).sum(1, dtype=np.float32))
    r2h, r2l = _split2((rtt * rtt).sum(1, dtype=np.float32))
    ones_q = np.ones(nq, np.float32)
    ones_r = np.ones(nr, np.float32)
    qaug = np.concatenate([
        qh.T, ql.T, qh.T, ql.T,
        ones_q[None], ones_q[None], q2h[None], q2l[None],
    ])  # [16, nq]
    raug = np.concatenate([
        2 * rh.T, 2 * rh.T, 2 * rl.T, 2 * rl.T,
        -r2h[None], -r2l[None], -ones_r[None], -ones_r[None],
    ])  # [16, nr]
    return {
        "qs": qaug.astype(NPBF16),
        "rs": raug.astype(NPBF16),
        "ident": np.eye(128, dtype=np.float32),
    }


def make_in_maps(pred, label):
    pred = np.asarray(pred, np.float32)
    label = np.asarray(label, np.float32)
    in_maps = []
    for b in range(B):
        for h in range(2):
            in_maps.append(prep_core(pred[b, h * NQ:(h + 1) * NQ], label[b]))
    return in_maps


def postprocess(results):
    # pred -> label direction: per-core device sums of sqrt(min d^2)
    sq_sum = sum(float(res["q_out"].sum(dtype=np.float64)) for res in results)
    # label -> pred: combine the two pred-halves per batch, then sqrt/sum
    ref_sum = 0.0
    for b in range(B):
        m = np.maximum(results[2 * b]["ref_out"], results[2 * b + 1]["ref_out"])
        ref_sum += float(np.sqrt(np.maximum(-m, 0.0)).sum(dtype=np.float64))
    return np.float32((sq_sum + ref_sum) / (B * N))


_PROGRAM = None


def _get_program():
    global _PROGRAM
    if _PROGRAM is None:
        _PROGRAM = build_program()
    return _PROGRAM


def run_on_hw(pred, label, trace=False):
    nc = _get_program()
    res = run_bass_kernel_spmd(nc, make_in_maps(pred, label),
                               list(range(NCORES)), trace=trace)
    return postprocess(res.results), res


def kernel(pred, label):
    out, _ = run_on_hw(pred, label)
    return out
